# revision 1
# baseline (speedup 1.0000x reference)
import numpy as np

# Self-contained kernel for nn_CorrBlock2 on 8 NeuronCores.
# Strategy: shard n_p (points) across 8 devices with shard_map; the only
# cross-point coupling (GroupNorm statistics) is done with jax.lax.psum.
#
# Key algebraic simplifications (exact, verified against the reference):
#  * ids=3, num_iters=12 -> adaptive branch taken; the per-level length
#    factors (r1_len/r2_len magnitudes, incl. r1_max/r2_max) cancel inside
#    round(sum(diff*v)/||v||), so d0/d1 are IDENTICAL for all 3 levels.
#    => one 9-bin histogram; w1 folds to W1r = sum of its three 9-col blocks.
#  * (u1, u2) is an orthonormal frame => no r1_max/r2_max all-reduce needed.
#  * dynamic_k = 24 - 2*min(3,10) = 18.

_RES = 3
_NUM_LEVELS = 3
_KNN = 24
_EPS = 1e-8

_N_CORES = 8

_compiled = None


def _build():
    global _compiled
    if _compiled is not None:
        return _compiled

    import jax
    import jax.numpy as jnp
    from jax.sharding import Mesh, PartitionSpec as P
    from jax.experimental.shard_map import shard_map
    from functools import partial

    devs = jax.devices()[:_N_CORES]
    mesh = Mesh(np.array(devs), ("p",))

    def shard_fn(coords, f1, f2, corr, xy2,
                 w1r, b1, g1, beta1, a1, w2, b2,
                 wk, bk, gk, betak, ak, wo, bo):
        # coords [b,S,2] f1,f2 [b,S,2] corr [b,S,K] xy2 [b,S,K,2]
        b, S, K = corr.shape
        n_p_global = S * _N_CORES

        r1n = jnp.sqrt(jnp.sum(f1 * f1, -1, keepdims=True))          # [b,S,1]
        f2n = jnp.sqrt(jnp.sum(f2 * f2, -1, keepdims=True))
        u1 = f1 / r1n
        cos1 = jnp.sum(f1 * f2, -1, keepdims=True) / (
            jnp.maximum(r1n, _EPS) * jnp.maximum(f2n, _EPS))
        r2p = f2 - cos1 * f2n * u1
        r2n = jnp.sqrt(jnp.sum(r2p * r2p, -1, keepdims=True))
        u2 = r2p / r2n

        diff = xy2 - coords[:, :, None, :]                           # [b,S,K,2]
        w1c = diff[..., 0] * u1[..., 0:1] + diff[..., 1] * u1[..., 1:2]  # [b,S,K]
        w2c = diff[..., 0] * u2[..., 0:1] + diff[..., 1] * u2[..., 1:2]

        d0 = jnp.round(w1c)
        d1 = jnp.round(w2c)
        valid = (jnp.abs(d0) <= 1.0) & (jnp.abs(d1) <= 1.0)
        cube = jnp.where(valid, (d0 + 1.0) * 3.0 + (d1 + 1.0), 9.0)
        onehot = (cube[..., None] == jnp.arange(9, dtype=cube.dtype)
                  ).astype(corr.dtype)                               # [b,S,K,9]
        h = jnp.einsum("bskj,bsk->bjs", onehot, corr)                # [b,9,S]
        cnt = jnp.clip(jnp.sum(onehot, axis=2), 1.0, float(n_p_global))
        feats = h / cnt.transpose(0, 2, 1)                           # [b,9,S]

        x = jnp.einsum("oc,bcs->bos", w1r, feats) + b1[None, :, None]  # [b,64,S]

        # knn branch (pre-normalization part, before the single collective)
        dist = w1c * w1c + w2c * w2c                                 # [b,S,K]
        _, nbrs = jax.lax.top_k(-dist, 18)                           # [b,S,18]
        kc = jnp.take_along_axis(corr, nbrs, axis=2)                 # [b,S,18]
        kxy = jnp.take_along_axis(xy2, nbrs[..., None], axis=2)      # [b,S,18,2]
        kdx = kxy[..., 0] - coords[..., 0:1]
        kdy = kxy[..., 1] - coords[..., 1:2]
        yfeat = jnp.stack([kc, kdx, kdy], axis=-1)                   # [b,S,18,3]
        yh = jnp.einsum("of,bskf->bsko", wk, yfeat) + bk             # [b,S,18,64]

        # GroupNorm stats for BOTH branches via ONE all-reduce.
        xs = jnp.sum(x, axis=2)                                      # [b,64]
        xs2 = jnp.sum(x * x, axis=2)
        ys = jnp.sum(yh, axis=(1, 2))                                # [b,64]
        ys2 = jnp.sum(yh * yh, axis=(1, 2))
        stats = jax.lax.psum(jnp.stack([xs, xs2, ys, ys2], 0), "p")  # [4,b,64]
        xs, xs2, ys, ys2 = stats[0], stats[1], stats[2], stats[3]

        cnt_x = 8.0 * n_p_global
        mu_g = jnp.sum(xs.reshape(b, 8, 8), -1) / cnt_x              # [b,8]
        var_g = jnp.sum(xs2.reshape(b, 8, 8), -1) / cnt_x - mu_g * mu_g
        inv_g = jax.lax.rsqrt(var_g + 1e-5)
        mu_c = jnp.repeat(mu_g, 8, axis=1)[:, :, None]
        inv_c = jnp.repeat(inv_g, 8, axis=1)[:, :, None]
        xn = (x - mu_c) * inv_c * g1[None, :, None] + beta1[None, :, None]
        xh = jnp.where(xn >= 0, xn, a1 * xn)
        vox = jnp.einsum("oc,bcs->bos", w2, xh) + b2[None, :, None]  # [b,64,S]

        cnt_y = 8.0 * n_p_global * 18.0
        muy_g = jnp.sum(ys.reshape(b, 8, 8), -1) / cnt_y
        vary_g = jnp.sum(ys2.reshape(b, 8, 8), -1) / cnt_y - muy_g * muy_g
        invy_g = jax.lax.rsqrt(vary_g + 1e-5)
        muy_c = jnp.repeat(muy_g, 8, axis=1)[:, None, None, :]
        invy_c = jnp.repeat(invy_g, 8, axis=1)[:, None, None, :]
        yn = (yh - muy_c) * invy_c * gk[None, None, None, :] + betak[None, None, None, :]
        yp = jnp.where(yn >= 0, yn, ak * yn)
        ymax = jnp.max(yp, axis=2)                                   # [b,S,64]
        out = vox + jnp.einsum("oc,bsc->bos", wo, ymax) + bo[None, :, None]
        return out                                                   # [b,64,S]

    pt = P(None, "p")
    ptk = P(None, "p", None)
    ptk2 = P(None, "p", None, None)
    rep = P()
    fn = shard_map(
        shard_fn, mesh=mesh,
        in_specs=(pt, pt, pt, ptk, ptk2,
                  rep, rep, rep, rep, rep, rep, rep,
                  rep, rep, rep, rep, rep, rep, rep),
        out_specs=P(None, None, "p"),
        check_rep=False,
    )
    jfn = jax.jit(fn)
    _compiled = (jax, jnp, mesh, jfn)
    return _compiled


def kernel(coords, all_delta_flow, truncated_corr, truncate_xy2,
           w1, b1, g1, beta1, a1, w2, b2,
           wk, bk, gk, betak, ak, wo, bo, num_iters, scale):
    jax, jnp, mesh, jfn = _build()

    ids = all_delta_flow.shape[0]
    ni = int(np.asarray(num_iters))
    adaptive = (ids >= 2) and (ids < ni - 2)
    assert adaptive, "kernel specialized for the adaptive branch (ids=3, num_iters=12)"
    dk = _KNN - 2 * min(ids, 10)
    assert dk == 18

    coords = np.asarray(coords, np.float32)
    f1 = np.asarray(all_delta_flow[-1], np.float32)
    f2 = np.asarray(all_delta_flow[-2], np.float32)
    corr = np.asarray(truncated_corr, np.float32)
    xy2 = np.asarray(truncate_xy2, np.float32)

    # fold the 3 identical levels into one [64,9] matrix
    w1 = np.asarray(w1, np.float32)
    w1r = w1[:, 0:9] + w1[:, 9:18] + w1[:, 18:27]

    args = (coords, f1, f2, corr, xy2,
            w1r, np.asarray(b1, np.float32), np.asarray(g1, np.float32),
            np.asarray(beta1, np.float32), np.asarray(a1, np.float32).reshape(()),
            np.asarray(w2, np.float32), np.asarray(b2, np.float32),
            np.asarray(wk, np.float32), np.asarray(bk, np.float32),
            np.asarray(gk, np.float32), np.asarray(betak, np.float32),
            np.asarray(ak, np.float32).reshape(()),
            np.asarray(wo, np.float32), np.asarray(bo, np.float32))
    out = jfn(*args)
    return np.asarray(jax.block_until_ready(out))



# revision 2
# speedup vs baseline: 2.1118x; 2.1118x over previous
import numpy as np

# Self-contained kernel for nn_CorrBlock2 on 8 NeuronCores.
# Strategy: shard n_p (points) across 8 devices with shard_map; the only
# cross-point coupling (GroupNorm statistics) is done with jax.lax.psum.
#
# Key algebraic simplifications (exact, verified against the reference):
#  * ids=3, num_iters=12 -> adaptive branch taken; the per-level length
#    factors (r1_len/r2_len magnitudes, incl. r1_max/r2_max) cancel inside
#    round(sum(diff*v)/||v||), so d0/d1 are IDENTICAL for all 3 levels.
#    => one 9-bin histogram; w1 folds to W1r = sum of its three 9-col blocks.
#  * (u1, u2) is an orthonormal frame => no r1_max/r2_max all-reduce needed,
#    and dist = diff_x^2 + diff_y^2 directly.
#  * dynamic_k = 24 - 2*min(3,10) = 18.
#  * GroupNorm affine + PReLU are monotone per channel, so the knn-branch
#    max over neighbors commutes: max_j prelu(gn(wk@g_j + bk)) =
#    prelu(gn_affine(max_j wk@g_j)). The [b,S,18,64] tensor never exists.
#  * GroupNorm stats of the knn branch follow from first/second moments
#    (m1, M2) of the gathered 3-vectors g: sum_c (wk_c@g+bk_c) and its
#    square are linear/quadratic forms in (m1, M2).

_RES = 3
_NUM_LEVELS = 3
_KNN = 24
_EPS = 1e-8

_N_CORES = 8

_compiled = None


def _build():
    global _compiled
    if _compiled is not None:
        return _compiled

    import jax
    import jax.numpy as jnp
    from jax.sharding import Mesh, PartitionSpec as P
    from jax.experimental.shard_map import shard_map

    devs = jax.devices()[:_N_CORES]
    mesh = Mesh(np.array(devs), ("p",))

    def shard_fn(coords, f1, f2, corr, xy2,
                 w1r, b1, g1, beta1, a1, w2, b2,
                 wk, bk, gk, betak, ak, wo, bo):
        # coords [b,S,2] f1,f2 [b,S,2] corr [b,S,K] xy2 [b,S,K,2]
        b, S, K = corr.shape
        n_p_global = S * _N_CORES

        r1n = jnp.sqrt(jnp.sum(f1 * f1, -1, keepdims=True))          # [b,S,1]
        f2n = jnp.sqrt(jnp.sum(f2 * f2, -1, keepdims=True))
        u1 = f1 / r1n
        cos1 = jnp.sum(f1 * f2, -1, keepdims=True) / (
            jnp.maximum(r1n, _EPS) * jnp.maximum(f2n, _EPS))
        r2p = f2 - cos1 * f2n * u1
        r2n = jnp.sqrt(jnp.sum(r2p * r2p, -1, keepdims=True))
        u2 = r2p / r2n

        dx = xy2[..., 0] - coords[:, :, None, 0]                     # [b,S,K]
        dy = xy2[..., 1] - coords[:, :, None, 1]

        w1c = dx * u1[..., 0:1] + dy * u1[..., 1:2]                  # [b,S,K]
        w2c = dx * u2[..., 0:1] + dy * u2[..., 1:2]

        d0 = jnp.round(w1c)
        d1 = jnp.round(w2c)
        valid = (jnp.abs(d0) <= 1.0) & (jnp.abs(d1) <= 1.0)
        cube = jnp.where(valid, d0 * 3.0 + d1 + 4.0, -1.0)           # [b,S,K]

        # 9-bin histogram without materializing the one-hot tensor
        feats = []
        for j in range(9):
            mf = (cube == float(j)).astype(corr.dtype)
            sj = jnp.sum(corr * mf, axis=-1)                         # [b,S]
            cj = jnp.sum(mf, axis=-1)
            feats.append(sj / jnp.clip(cj, 1.0, float(n_p_global)))
        feats = jnp.stack(feats, axis=1)                             # [b,9,S]

        x = jnp.einsum("oc,bcs->bos", w1r, feats) + b1[None, :, None]  # [b,64,S]

        # knn: top-18 by distance, gather (corr, dx, dy) jointly
        dist = dx * dx + dy * dy                                     # [b,S,K]
        _, nbrs = jax.lax.top_k(-dist, 18)                           # [b,S,18]
        F = jnp.stack([corr, dx, dy], axis=-1)                       # [b,S,K,3]
        g = jnp.take_along_axis(F, nbrs[..., None], axis=2)          # [b,S,18,3]

        # GroupNorm stats for BOTH branches via ONE all-reduce.
        xs = jnp.sum(x, axis=2)                                      # [b,64]
        xs2 = jnp.sum(x * x, axis=2)
        m1 = jnp.sum(g, axis=(1, 2))                                 # [b,3]
        M2 = jnp.einsum("bsjf,bsjg->bfg", g, g).reshape(b, 9)        # [b,9]
        stats = jnp.concatenate([xs, xs2, m1, M2], axis=1)           # [b,140]
        stats = jax.lax.psum(stats, "p")
        xs, xs2 = stats[:, 0:64], stats[:, 64:128]
        m1, M2 = stats[:, 128:131], stats[:, 131:140].reshape(b, 3, 3)

        # x-branch GroupNorm (8 groups) + PReLU + conv
        cnt_x = 8.0 * n_p_global
        mu_g = jnp.sum(xs.reshape(b, 8, 8), -1) / cnt_x              # [b,8]
        var_g = jnp.sum(xs2.reshape(b, 8, 8), -1) / cnt_x - mu_g * mu_g
        inv_g = jax.lax.rsqrt(var_g + 1e-5)
        mu_c = jnp.repeat(mu_g, 8, axis=1)[:, :, None]
        inv_c = jnp.repeat(inv_g, 8, axis=1)[:, :, None]
        xn = (x - mu_c) * inv_c * g1[None, :, None] + beta1[None, :, None]
        xh = jnp.where(xn >= 0, xn, a1 * xn)
        vox = jnp.einsum("oc,bcs->bos", w2, xh) + b2[None, :, None]  # [b,64,S]

        # y-branch stats from (m1, M2): per channel c,
        #   sum_j yh_jc = wk_c @ m1 + N*bk_c
        #   sum_j yh_jc^2 = wk_c @ M2 @ wk_c + 2 bk_c wk_c@m1 + N bk_c^2
        N_tot = float(n_p_global) * 18.0                             # per batch
        wm = jnp.einsum("cf,bf->bc", wk, m1)                         # [b,64]
        sum_y = wm + N_tot * bk[None, :]
        quad = jnp.einsum("cf,bfg,cg->bc", wk, M2, wk)
        sum_y2 = quad + 2.0 * bk[None, :] * wm + N_tot * bk[None, :] ** 2
        cnt_y = 8.0 * N_tot
        muy_g = jnp.sum(sum_y.reshape(b, 8, 8), -1) / cnt_y          # [b,8]
        vary_g = jnp.sum(sum_y2.reshape(b, 8, 8), -1) / cnt_y - muy_g * muy_g
        invy_g = jax.lax.rsqrt(vary_g + 1e-5)
        muy_c = jnp.repeat(muy_g, 8, axis=1)[:, None, :]             # [b,1,64]
        invy_c = jnp.repeat(invy_g, 8, axis=1)[:, None, :]

        # knn max via monotone commutation: M_c = max_j wk_c @ g_j
        M = g[:, :, 0, :] @ wk.T                                     # [b,S,64]
        for j in range(1, 18):
            M = jnp.maximum(M, g[:, :, j, :] @ wk.T)
        M = M + bk[None, None, :]
        yn = (M - muy_c) * invy_c * gk[None, None, :] + betak[None, None, :]
        yp = jnp.where(yn >= 0, yn, ak * yn)                         # [b,S,64]
        out = vox + jnp.einsum("oc,bsc->bos", wo, yp) + bo[None, :, None]
        return out                                                   # [b,64,S]

    pt = P(None, "p")
    ptk = P(None, "p", None)
    ptk2 = P(None, "p", None, None)
    rep = P()
    fn = shard_map(
        shard_fn, mesh=mesh,
        in_specs=(pt, pt, pt, ptk, ptk2,
                  rep, rep, rep, rep, rep, rep, rep,
                  rep, rep, rep, rep, rep, rep, rep),
        out_specs=P(None, None, "p"),
        check_rep=False,
    )
    jfn = jax.jit(fn)
    _compiled = (jax, jnp, mesh, jfn)
    return _compiled


def kernel(coords, all_delta_flow, truncated_corr, truncate_xy2,
           w1, b1, g1, beta1, a1, w2, b2,
           wk, bk, gk, betak, ak, wo, bo, num_iters, scale):
    jax, jnp, mesh, jfn = _build()

    ids = all_delta_flow.shape[0]
    ni = int(np.asarray(num_iters))
    adaptive = (ids >= 2) and (ids < ni - 2)
    assert adaptive, "kernel specialized for the adaptive branch (ids=3, num_iters=12)"
    dk = _KNN - 2 * min(ids, 10)
    assert dk == 18

    coords = np.asarray(coords, np.float32)
    f1 = np.asarray(all_delta_flow[-1], np.float32)
    f2 = np.asarray(all_delta_flow[-2], np.float32)
    corr = np.asarray(truncated_corr, np.float32)
    xy2 = np.asarray(truncate_xy2, np.float32)

    # fold the 3 identical levels into one [64,9] matrix
    w1 = np.asarray(w1, np.float32)
    w1r = w1[:, 0:9] + w1[:, 9:18] + w1[:, 18:27]

    args = (coords, f1, f2, corr, xy2,
            w1r, np.asarray(b1, np.float32), np.asarray(g1, np.float32),
            np.asarray(beta1, np.float32), np.asarray(a1, np.float32).reshape(()),
            np.asarray(w2, np.float32), np.asarray(b2, np.float32),
            np.asarray(wk, np.float32), np.asarray(bk, np.float32),
            np.asarray(gk, np.float32), np.asarray(betak, np.float32),
            np.asarray(ak, np.float32).reshape(()),
            np.asarray(wo, np.float32), np.asarray(bo, np.float32))
    out = jfn(*args)
    return np.asarray(jax.block_until_ready(out))
